# revision 22
# baseline (speedup 1.0000x reference)
"""CTC loss (nn_CTCLoss) on 8 Trainium2 NeuronCores.

Strategy (data parallel over batch B, as per the sharding hint):
  - Host: build the extended-label table tp (blanks interleaved), gather the
    per-extended-label log-probs lp_tp[b, s, t] = log_probs[t, b, tp[b, s]],
    and the skip-transition mask. Shard batch across 8 cores (8 samples each).
  - Device (per core): exp on the ACT engine, then the T-step CTC forward DP.
    The DP is reorganized row-by-row over the 65 extended-label positions:
    each row is one hardware `tensor_tensor_scan` along the time axis
        state[t] = (cross[t-1] + state[t-1]) * pt[s, t]
    where cross = alpha[s-1] + where(mask, alpha[s-2], 0) comes from the two
    previously computed rows (one fused scalar_tensor_tensor op; the where is
    an exact min(x, {0,+inf}) per-partition trick, safe for inf without
    creating 0*inf NaNs). Instead of freezing alpha at t >= input_length, the
    full (unfrozen) trajectory x[b, s, t] is written out; the value at
    t = input_length-1 equals the reference's frozen final alpha.
  - Host: read alpha at t=il-1, loss = -log(a[2*tl] + a[2*tl-1]), mean(loss/tl).
"""

import numpy as np

T, B, C, S = 256, 64, 6000, 32
L = 2 * S + 1  # 65 extended labels (blank interleaved)
BLANK = 0
N_CORES = 8
P = B // N_CORES  # samples per core

_CACHE = {}

# test-harness knobs (harness calls kernel() with defaults untouched)
TRACE = False
LAST_RESULT = None


def _build_program():
    import bass_rust
    import concourse.bass as bass
    import concourse.mybir as mybir
    from concourse import tile

    class SingleWaitTileContext(tile.TileContext):
        """TileContext whose epilogue drain never carries more than one sync
        wait: this target's codegen rejects any instruction with 2+ waits, and
        the stock epilogue puts the whole global clock on one drain. Absorb
        each outstanding semaphore tick with its own SP nop first."""

        def _drain_and_barrier(self, tick_clock, wait_clock):
            g = tick_clock.global_clock
            for proc in range(len(g)):
                if g[proc] > 0:
                    nop = self.nc.sync.nop()
                    pv = bass_rust.VectorClock()
                    pv.require_at_least(proc, g[proc])
                    wait_clock.add_sem_waits(
                        nop.ins, bass_rust.ScopedClock({None: pv})
                    )
            # stock epilogue, but with a wait-free drain: the nops above
            # already hold SP until the full global clock is reached
            self.nc.sync.drain()
            self.nc.all_engine_barrier()
            assert self.sems is not None
            popped = self.nc._tile_sem_poison_stack.pop()
            assert popped is self._sem_poison
            self.nc.clear_and_free_semaphores(
                list(self.sems.allocated().values())
            )
            self.nc.all_engine_barrier()

    f32 = mybir.dt.float32
    Alu = mybir.AluOpType
    Act = mybir.ActivationFunctionType

    nc = bass.Bass()
    lp_in = nc.dram_tensor("lp_tp", [P, L, T], f32, kind="ExternalInput")
    ms_in = nc.dram_tensor("msel", [P, L], f32, kind="ExternalInput")
    x_out = nc.dram_tensor("x_out", [P, L, T], f32, kind="ExternalOutput")

    CH = 13  # rows per compute (fence) chunk (65 = 5 * 13)
    IN_CH = 22  # rows per input-DMA chunk (3 chunks: 22+22+21)
    # 1 msel + 3 input + 4 output DMAs = 8 total: every DMA gets a fresh
    # DMAHW lane semaphore, and the input ring has one slot per chunk
    # (bufs=3, no reuse), so no DMA ever needs a second sync wait (all
    # instructions on this target have a single sync-wait slot).

    # The TensorScalarPtr (scalar_tensor_tensor / tensor_tensor_scan) HW
    # encoding has a single sync-wait slot, so every cross-engine dependency
    # of those ops must be absorbed by a preceding non-TSP DVE instruction.
    # The col-0 "zero init" TensorTensor ops double as those fences: each
    # chunk's TT reads the chunk's exp'd pt rows (so it carries the ACT wait)
    # and writes the alpha[t=0] zeros that every scan in the chunk reads.
    with SingleWaitTileContext(nc) as tc:
        with (
            tc.tile_pool(name="main", bufs=1) as pool,
            tc.tile_pool(name="ring", bufs=3) as ring,
            tc.tile_pool(name="vp", bufs=4) as vp,
        ):
            pt = pool.tile([P, L, T], f32)      # exp'd emission probs per row
            x = pool.tile([P, L, T], f32)       # alpha trajectories
            ms_raw = pool.tile([P, L], f32)     # skip mask as {0, +inf} (DMA)
            ms = pool.tile([P, L], f32)         # DVE-owned copy (wait absorber)
            zeros = pool.tile([P, T], f32)

            # col 0 of `zeros` is written by the pt-dependent TT below instead,
            # so that TT carries only the ACT wait (DVE ops have ONE wait slot)
            nc.vector.memset(zeros[:, 1:T], 0.0)
            nc.vector.memset(x[:, L - 1 : L, 0:1], 0.0)  # row 64 alpha0
            nc.sync.dma_start(out=ms_raw[:], in_=ms_in[:])
            nc.vector.tensor_copy(ms[:], ms_raw[:])

            # stream log-probs in row chunks, exp per row on ACT. pt is only
            # ever ACT-written, so downstream DVE ops never need a DMA wait
            # (DVE instructions have a single sync-wait slot).
            for c0 in range(0, L, IN_CH):
                c1 = min(c0 + IN_CH, L)
                lp_chunk = ring.tile([P, IN_CH, T], f32, tag="lp")
                nc.sync.dma_start(
                    out=lp_chunk[:, 0 : c1 - c0, :], in_=lp_in[:, c0:c1, :]
                )
                for s in range(c0, c1):
                    nc.scalar.activation(
                        pt[:, s, :], lp_chunk[:, s - c0, :], Act.Exp
                    )

            for c0 in range(0, L, CH):
                c1 = min(c0 + CH, L)  # chunk rows [c0, c1)
                # --- fence + alpha[t=0] init for this chunk ---
                if c0 == 0:
                    # zeros col 0 rewritten as a pt-dependent zero. Reading
                    # pt row 12 makes the first DVE op of the scan chain wait
                    # for ALL of chunk 0's exps, so every scan in the chunk
                    # inherits the ACT clock transitively (scan rows 0-2
                    # don't read the chunk fence below).
                    nc.vector.tensor_tensor(
                        out=zeros[:, 0:1],
                        in0=pt[:, CH - 1, 0:1],
                        in1=pt[:, CH - 1, 0:1],
                        op=Alu.subtract,
                    )
                    # alpha0 of rows 0,1 is pt[t=0]; carries the ACT>=2 wait
                    # that scan row 1 needs
                    nc.vector.tensor_copy(x[:, 0:2, 0:1], pt[:, 0:2, 0:1])
                    lo = 2
                else:
                    lo = c0 - 1
                # zero alpha[t=0] for rows [lo, c1-1): every scan row s in
                # this chunk reads row s-1 from exactly this range, so this TT
                # fences the chunk. Ranges of successive fences are disjoint
                # (no WAW) so each carries only its ACT wait. Reading this
                # chunk's last pt row makes it wait for all the chunk's exps.
                n = c1 - 1 - lo
                nc.vector.tensor_tensor(
                    out=x[:, lo : c1 - 1, 0:1],
                    in0=pt[:, c1 - n : c1, 0:1],
                    in1=pt[:, c1 - n : c1, 0:1],
                    op=Alu.subtract,
                )
                # --- scans for this chunk ---
                for s in range(c0, c1):
                    if s == 0:
                        data0 = zeros[:, 0 : T - 1]
                        init = pt[:, 0, 0:1]
                    elif s == 1:
                        data0 = x[:, 0, 0 : T - 1]
                        init = pt[:, 1, 0:1]
                    else:
                        init = 0.0
                        if s % 2 == 0:
                            # blank rows can never take the skip transition
                            data0 = x[:, s - 1, 0 : T - 1]
                        else:
                            v = vp.tile([P, T - 1], f32, tag="v")
                            nc.vector.scalar_tensor_tensor(
                                out=v[:],
                                in0=x[:, s - 2, 0 : T - 1],
                                scalar=ms[:, s - 2 : s - 1],
                                in1=x[:, s - 1, 0 : T - 1],
                                op0=Alu.min,
                                op1=Alu.add,
                            )
                            data0 = v[:]
                    nc.vector.tensor_tensor_scan(
                        out=x[:, s, 1:T],
                        data0=data0,
                        data1=pt[:, s, 1:T],
                        initial=init,
                        op0=Alu.add,
                        op1=Alu.mult,
                    )
            # stream trajectories out in 4 chunks. Col 0 is skipped: the host
            # only reads t = il-1 >= 127, and skipping it keeps the out-DMA
            # read ranges disjoint from the col-0 fence writes.
            for o0, o1 in ((0, 17), (17, 33), (33, 49), (49, 65)):
                nc.sync.dma_start(out=x_out[:, o0:o1, 1:T], in_=x[:, o0:o1, 1:T])

    return nc


def _host_prep(log_probs, targets, input_lengths, target_lengths):
    lp = np.asarray(log_probs, dtype=np.float32)
    tg = np.asarray(targets).astype(np.int64)
    tl = np.asarray(target_lengths).astype(np.int64)

    j = np.arange(L)
    idx = np.clip(j // 2, 0, S - 1)
    odd = (j % 2) == 1
    valid = odd[None, :] & ((j // 2)[None, :] < tl[:, None])
    tp = np.where(valid, tg[:, idx], BLANK)  # (B, L)
    mask3 = tp[:, :-2] != tp[:, 2:]  # (B, L-2)

    # gather lp_tp[t, b, s] = lp[t, b, tp[b, s]] then lay out (B, L, T)
    g = lp[:, np.arange(B)[:, None], tp]  # (T, B, L)
    lp_tp = np.ascontiguousarray(g.transpose(1, 2, 0))  # (B, L, T)

    msel = np.zeros((B, L), np.float32)
    msel[:, : L - 2] = np.where(mask3, np.float32(np.inf), np.float32(0.0))
    return lp_tp, msel, tp


def kernel(log_probs, targets, input_lengths, target_lengths):
    from concourse.bass_utils import run_bass_kernel_spmd

    il = np.asarray(input_lengths).astype(np.int64)
    tl = np.asarray(target_lengths).astype(np.int64)
    lp_tp, msel, _ = _host_prep(log_probs, targets, input_lengths, target_lengths)

    if "nc" not in _CACHE:
        _CACHE["nc"] = _build_program()
    nc = _CACHE["nc"]

    in_maps = [
        {
            "lp_tp": np.ascontiguousarray(lp_tp[c * P : (c + 1) * P]),
            "msel": np.ascontiguousarray(msel[c * P : (c + 1) * P]),
        }
        for c in range(N_CORES)
    ]
    res = run_bass_kernel_spmd(nc, in_maps, list(range(N_CORES)), trace=TRACE)
    global LAST_RESULT
    LAST_RESULT = res
    x = np.concatenate(
        [res.results[c]["x_out"] for c in range(N_CORES)], axis=0
    )  # (B, L, T)

    # readout: alpha at t = il-1 equals the reference's frozen final alpha
    bi = np.arange(B)
    af = x[bi[:, None], np.arange(L)[None, :], (il - 1)[:, None]]  # (B, L)
    end = 2 * tl
    ae = af[bi, end].astype(np.float32)
    ae1 = af[bi, end - 1].astype(np.float32)
    with np.errstate(divide="ignore", over="ignore"):
        loss = (-np.log(ae + ae1)).astype(np.float32)
        out = np.float32(np.mean((loss / tl.astype(np.float32)).astype(np.float32)))
    return np.array(out, dtype=np.float32)


# revision 26
# speedup vs baseline: 1.0494x; 1.0494x over previous
"""CTC loss (nn_CTCLoss) on 8 Trainium2 NeuronCores.

Strategy (data parallel over batch B, as per the sharding hint):
  - Host: build the extended-label table tp (blanks interleaved), gather the
    per-extended-label log-probs lp_tp[b, s, t] = log_probs[t, b, tp[b, s]],
    and the skip-transition mask. Shard batch across 8 cores (8 samples each).
  - Device (per core): exp on the ACT engine, then the T-step CTC forward DP.
    The DP is reorganized row-by-row over the 65 extended-label positions:
    each row is one hardware `tensor_tensor_scan` along the time axis
        state[t] = (cross[t-1] + state[t-1]) * pt[s, t]
    where cross = alpha[s-1] + where(mask, alpha[s-2], 0) comes from the two
    previously computed rows (one fused scalar_tensor_tensor op; the where is
    an exact min(x, {0,+inf}) per-partition trick, safe for inf without
    creating 0*inf NaNs). Instead of freezing alpha at t >= input_length, the
    full (unfrozen) trajectory x[b, s, t] is written out; the value at
    t = input_length-1 equals the reference's frozen final alpha.
  - Host: read alpha at t=il-1, loss = -log(a[2*tl] + a[2*tl-1]), mean(loss/tl).
"""

import numpy as np

T, B, C, S = 256, 64, 6000, 32
L = 2 * S + 1  # 65 extended labels (blank interleaved)
BLANK = 0
N_CORES = 8
P = B // N_CORES  # samples per core

_CACHE = {}

# test-harness knobs (harness calls kernel() with defaults untouched)
TRACE = False
LAST_RESULT = None


def _build_program(modes=None):
    import bass_rust
    import concourse.bass as bass
    import concourse.mybir as mybir
    from concourse import tile

    # modes[s] for odd rows s>=3: 'stt' = per-sample masked skip (min trick),
    # 'add' = all samples allow the skip (plain add), 'skip' = no sample
    # allows it (copy previous row only). Derived from the actual msel input
    # so the compiled program matches the dataset; defaults to the fully
    # general 'stt' everywhere.
    if modes is None:
        modes = ["stt"] * L

    class SingleWaitTileContext(tile.TileContext):
        """TileContext whose epilogue drain never carries more than one sync
        wait: this target's codegen rejects any instruction with 2+ waits, and
        the stock epilogue puts the whole global clock on one drain. Absorb
        each outstanding semaphore tick with its own SP nop first."""

        def _drain_and_barrier(self, tick_clock, wait_clock):
            g = tick_clock.global_clock
            for proc in range(len(g)):
                if g[proc] > 0:
                    nop = self.nc.sync.nop()
                    pv = bass_rust.VectorClock()
                    pv.require_at_least(proc, g[proc])
                    wait_clock.add_sem_waits(
                        nop.ins, bass_rust.ScopedClock({None: pv})
                    )
            # stock epilogue, but with a wait-free drain: the nops above
            # already hold SP until the full global clock is reached
            self.nc.sync.drain()
            self.nc.all_engine_barrier()
            assert self.sems is not None
            popped = self.nc._tile_sem_poison_stack.pop()
            assert popped is self._sem_poison
            self.nc.clear_and_free_semaphores(
                list(self.sems.allocated().values())
            )
            self.nc.all_engine_barrier()

    f32 = mybir.dt.float32
    Alu = mybir.AluOpType
    Act = mybir.ActivationFunctionType

    nc = bass.Bass()
    lp_in = nc.dram_tensor("lp_tp", [P, L, T], f32, kind="ExternalInput")
    ms_in = nc.dram_tensor("msel", [P, L], f32, kind="ExternalInput")
    x_out = nc.dram_tensor("x_out", [P, L, T], f32, kind="ExternalOutput")

    CH = 5  # rows per compute (fence) chunk (65 = 13 * 5)
    IN_CH = 22  # rows per input-DMA chunk (3 chunks: 22+22+21)
    # 1 msel + 3 input + 4 output DMAs = 8 total: every DMA gets a fresh
    # DMAHW lane semaphore, and the input ring has one slot per chunk
    # (bufs=3, no reuse), so no DMA ever needs a second sync wait (all
    # instructions on this target have a single sync-wait slot).

    # The TensorScalarPtr (scalar_tensor_tensor / tensor_tensor_scan) HW
    # encoding has a single sync-wait slot, so every cross-engine dependency
    # of those ops must be absorbed by a preceding non-TSP DVE instruction.
    # The col-0 "zero init" TensorTensor ops double as those fences: each
    # chunk's TT reads the chunk's exp'd pt rows (so it carries the ACT wait)
    # and writes the alpha[t=0] zeros that every scan in the chunk reads.
    with SingleWaitTileContext(nc) as tc:
        with (
            tc.tile_pool(name="main", bufs=1) as pool,
            tc.tile_pool(name="ring", bufs=3) as ring,
            tc.tile_pool(name="vp", bufs=4) as vp,
        ):
            pt = pool.tile([P, L, T], f32)      # exp'd emission probs per row
            x = pool.tile([P, L, T], f32)       # alpha trajectories
            ms_raw = pool.tile([P, L], f32)     # skip mask as {0, +inf} (DMA)
            ms = pool.tile([P, L], f32)         # DVE-owned copy (wait absorber)
            zeros = pool.tile([P, T], f32)

            # col 0 of `zeros` is written by the pt-dependent TT below instead,
            # so that TT carries only the ACT wait (DVE ops have ONE wait slot)
            nc.vector.memset(zeros[:, 1:T], 0.0)
            nc.vector.memset(x[:, L - 1 : L, 0:1], 0.0)  # row 64 alpha0
            nc.sync.dma_start(out=ms_raw[:], in_=ms_in[:])
            nc.vector.tensor_copy(ms[:], ms_raw[:])

            # stream log-probs in row chunks, exp per row on ACT. pt is only
            # ever ACT-written, so downstream DVE ops never need a DMA wait
            # (DVE instructions have a single sync-wait slot).
            for c0 in range(0, L, IN_CH):
                c1 = min(c0 + IN_CH, L)
                lp_chunk = ring.tile([P, IN_CH, T], f32, tag="lp")
                nc.sync.dma_start(
                    out=lp_chunk[:, 0 : c1 - c0, :], in_=lp_in[:, c0:c1, :]
                )
                for s in range(c0, c1):
                    nc.scalar.activation(
                        pt[:, s, :], lp_chunk[:, s - c0, :], Act.Exp
                    )

            for c0 in range(0, L, CH):
                c1 = min(c0 + CH, L)  # chunk rows [c0, c1)
                # --- fence + alpha[t=0] init for this chunk ---
                if c0 == 0:
                    # zeros col 0 rewritten as a pt-dependent zero. Reading
                    # pt row 12 makes the first DVE op of the scan chain wait
                    # for ALL of chunk 0's exps, so every scan in the chunk
                    # inherits the ACT clock transitively (scan rows 0-2
                    # don't read the chunk fence below).
                    nc.vector.tensor_tensor(
                        out=zeros[:, 0:1],
                        in0=pt[:, CH - 1, 0:1],
                        in1=pt[:, CH - 1, 0:1],
                        op=Alu.subtract,
                    )
                    # alpha0 of rows 0,1 is pt[t=0]; carries the ACT>=2 wait
                    # that scan row 1 needs
                    nc.vector.tensor_copy(x[:, 0:2, 0:1], pt[:, 0:2, 0:1])
                    lo = 2
                else:
                    lo = c0 - 1
                # zero alpha[t=0] for rows [lo, c1-1): every scan row s in
                # this chunk reads row s-1 from exactly this range, so this TT
                # fences the chunk. Ranges of successive fences are disjoint
                # (no WAW) so each carries only its ACT wait. Reading this
                # chunk's last pt row makes it wait for all the chunk's exps.
                n = c1 - 1 - lo
                nc.vector.tensor_tensor(
                    out=x[:, lo : c1 - 1, 0:1],
                    in0=pt[:, c1 - n : c1, 0:1],
                    in1=pt[:, c1 - n : c1, 0:1],
                    op=Alu.subtract,
                )
                # --- scans for this chunk ---
                for s in range(c0, c1):
                    if s == 0:
                        data0 = zeros[:, 0 : T - 1]
                        init = pt[:, 0, 0:1]
                    elif s == 1:
                        data0 = x[:, 0, 0 : T - 1]
                        init = pt[:, 1, 0:1]
                    else:
                        init = 0.0
                        if s % 2 == 0 or modes[s] == "skip":
                            # blank rows (and odd rows whose skip is masked
                            # for every sample) take no skip transition
                            data0 = x[:, s - 1, 0 : T - 1]
                        elif modes[s] == "add":
                            # every sample allows the skip: plain add
                            v = vp.tile([P, T - 1], f32, tag="v")
                            nc.vector.tensor_tensor(
                                out=v[:],
                                in0=x[:, s - 2, 0 : T - 1],
                                in1=x[:, s - 1, 0 : T - 1],
                                op=Alu.add,
                            )
                            data0 = v[:]
                        else:
                            v = vp.tile([P, T - 1], f32, tag="v")
                            nc.vector.scalar_tensor_tensor(
                                out=v[:],
                                in0=x[:, s - 2, 0 : T - 1],
                                scalar=ms[:, s - 2 : s - 1],
                                in1=x[:, s - 1, 0 : T - 1],
                                op0=Alu.min,
                                op1=Alu.add,
                            )
                            data0 = v[:]
                    nc.vector.tensor_tensor_scan(
                        out=x[:, s, 1:T],
                        data0=data0,
                        data1=pt[:, s, 1:T],
                        initial=init,
                        op0=Alu.add,
                        op1=Alu.mult,
                    )
            # stream trajectories out in 4 chunks. Col 0 is skipped: the host
            # only reads t = il-1 >= 127, and skipping it keeps the out-DMA
            # read ranges disjoint from the col-0 fence writes.
            for o0, o1 in ((0, 17), (17, 33), (33, 49), (49, 65)):
                nc.sync.dma_start(out=x_out[:, o0:o1, 1:T], in_=x[:, o0:o1, 1:T])

    return nc


def _host_prep(log_probs, targets, input_lengths, target_lengths):
    lp = np.asarray(log_probs, dtype=np.float32)
    tg = np.asarray(targets).astype(np.int64)
    tl = np.asarray(target_lengths).astype(np.int64)

    j = np.arange(L)
    idx = np.clip(j // 2, 0, S - 1)
    odd = (j % 2) == 1
    valid = odd[None, :] & ((j // 2)[None, :] < tl[:, None])
    tp = np.where(valid, tg[:, idx], BLANK)  # (B, L)
    mask3 = tp[:, :-2] != tp[:, 2:]  # (B, L-2)

    # gather lp_tp[t, b, s] = lp[t, b, tp[b, s]] then lay out (B, L, T)
    g = lp[:, np.arange(B)[:, None], tp]  # (T, B, L)
    lp_tp = np.ascontiguousarray(g.transpose(1, 2, 0))  # (B, L, T)

    msel = np.zeros((B, L), np.float32)
    msel[:, : L - 2] = np.where(mask3, np.float32(np.inf), np.float32(0.0))
    return lp_tp, msel, tp


def kernel(log_probs, targets, input_lengths, target_lengths):
    from concourse.bass_utils import run_bass_kernel_spmd

    il = np.asarray(input_lengths).astype(np.int64)
    tl = np.asarray(target_lengths).astype(np.int64)
    lp_tp, msel, _ = _host_prep(log_probs, targets, input_lengths, target_lengths)

    # specialize odd rows by the dataset's skip-mask pattern (across ALL
    # samples, since one SPMD program serves every core)
    modes = ["n"] * L
    for s in range(3, L, 2):
        col = msel[:, s - 2]
        if np.all(np.isposinf(col)):
            modes[s] = "add"
        elif np.all(col == 0.0):
            modes[s] = "skip"
        else:
            modes[s] = "stt"
    key = "".join(m[0] for m in modes)
    if key not in _CACHE:
        _CACHE[key] = _build_program(modes)
    nc = _CACHE[key]

    in_maps = [
        {
            "lp_tp": np.ascontiguousarray(lp_tp[c * P : (c + 1) * P]),
            "msel": np.ascontiguousarray(msel[c * P : (c + 1) * P]),
        }
        for c in range(N_CORES)
    ]
    res = run_bass_kernel_spmd(nc, in_maps, list(range(N_CORES)), trace=TRACE)
    global LAST_RESULT
    LAST_RESULT = res
    x = np.concatenate(
        [res.results[c]["x_out"] for c in range(N_CORES)], axis=0
    )  # (B, L, T)

    # readout: alpha at t = il-1 equals the reference's frozen final alpha
    bi = np.arange(B)
    af = x[bi[:, None], np.arange(L)[None, :], (il - 1)[:, None]]  # (B, L)
    end = 2 * tl
    ae = af[bi, end].astype(np.float32)
    ae1 = af[bi, end - 1].astype(np.float32)
    with np.errstate(divide="ignore", over="ignore"):
        loss = (-np.log(ae + ae1)).astype(np.float32)
        out = np.float32(np.mean((loss / tl.astype(np.float32)).astype(np.float32)))
    return np.array(out, dtype=np.float32)


# revision 31
# speedup vs baseline: 1.1013x; 1.0494x over previous
"""CTC loss (nn_CTCLoss) on 8 Trainium2 NeuronCores.

Strategy (data parallel over batch B, as per the sharding hint):
  - Host: build the extended-label table tp (blanks interleaved), gather the
    per-extended-label log-probs lp_tp[b, s, t] = log_probs[t, b, tp[b, s]],
    and the skip-transition mask. Shard batch across 8 cores (8 samples each).
  - Device (per core): exp on the ACT engine, then the T-step CTC forward DP.
    The DP is reorganized row-by-row over the 65 extended-label positions:
    each row is one hardware `tensor_tensor_scan` along the time axis
        state[t] = (cross[t-1] + state[t-1]) * pt[s, t]
    where cross = alpha[s-1] + where(mask, alpha[s-2], 0) comes from the two
    previously computed rows (one fused scalar_tensor_tensor op; the where is
    an exact min(x, {0,+inf}) per-partition trick, safe for inf without
    creating 0*inf NaNs). Instead of freezing alpha at t >= input_length, the
    full (unfrozen) trajectory x[b, s, t] is written out; the value at
    t = input_length-1 equals the reference's frozen final alpha.
  - Host: read alpha at t=il-1, loss = -log(a[2*tl] + a[2*tl-1]), mean(loss/tl).
"""

import numpy as np

T, B, C, S = 256, 64, 6000, 32
L = 2 * S + 1  # 65 extended labels (blank interleaved)
BLANK = 0
N_CORES = 8
P = B // N_CORES  # samples per core

_CACHE = {}

# test-harness knobs (harness calls kernel() with defaults untouched)
TRACE = False
LAST_RESULT = None


def _build_program(modes=None):
    import bass_rust
    import concourse.bass as bass
    import concourse.mybir as mybir
    from concourse import tile

    # modes[s] for odd rows s>=3: 'stt' = per-sample masked skip (min trick),
    # 'add' = all samples allow the skip (plain add), 'skip' = no sample
    # allows it (copy previous row only). Derived from the actual msel input
    # so the compiled program matches the dataset; defaults to the fully
    # general 'stt' everywhere.
    if modes is None:
        modes = ["stt"] * L

    class SingleWaitTileContext(tile.TileContext):
        """TileContext whose epilogue drain never carries more than one sync
        wait: this target's codegen rejects any instruction with 2+ waits, and
        the stock epilogue puts the whole global clock on one drain. Absorb
        each outstanding semaphore tick with its own SP nop first."""

        def _drain_and_barrier(self, tick_clock, wait_clock):
            g = tick_clock.global_clock
            for proc in range(len(g)):
                if g[proc] > 0:
                    nop = self.nc.sync.nop()
                    pv = bass_rust.VectorClock()
                    pv.require_at_least(proc, g[proc])
                    wait_clock.add_sem_waits(
                        nop.ins, bass_rust.ScopedClock({None: pv})
                    )
            # stock epilogue, but with a wait-free drain: the nops above
            # already hold SP until the full global clock is reached
            self.nc.sync.drain()
            self.nc.all_engine_barrier()
            assert self.sems is not None
            popped = self.nc._tile_sem_poison_stack.pop()
            assert popped is self._sem_poison
            self.nc.clear_and_free_semaphores(
                list(self.sems.allocated().values())
            )
            self.nc.all_engine_barrier()

    f32 = mybir.dt.float32
    Alu = mybir.AluOpType
    Act = mybir.ActivationFunctionType

    nc = bass.Bass()
    # input rows are [log-probs(T) | msel] packed: col T carries the skip
    # mask as {0, +inf} so no separate mask DMA is needed
    lp_in = nc.dram_tensor("lp_tp", [P, L, T + 1], f32, kind="ExternalInput")
    x_out = nc.dram_tensor("x_out", [P, L, T], f32, kind="ExternalOutput")

    CH = 5  # rows per compute (fence) chunk (65 = 13 * 5)
    IN_CHUNKS = ((0, 5), (5, 22), (22, 44), (44, 65))
    # 4 input + 4 output DMAs = 8 total: every DMA gets a fresh DMAHW lane
    # semaphore, and each input chunk has its own tile (no slot reuse), so
    # no DMA ever needs a second sync wait (every instruction on this
    # target has a single sync-wait slot). The first input chunk is small
    # so the scan chain starts early.

    # The TensorScalarPtr (scalar_tensor_tensor / tensor_tensor_scan) HW
    # encoding has a single sync-wait slot, so every cross-engine dependency
    # of those ops must be absorbed by a preceding non-TSP DVE instruction.
    # The col-0 "zero init" TensorTensor ops double as those fences: each
    # chunk's TT reads the chunk's exp'd pt rows (so it carries the ACT wait)
    # and writes the alpha[t=0] zeros that every scan in the chunk reads.
    with SingleWaitTileContext(nc) as tc:
        with (
            tc.tile_pool(name="main", bufs=1) as pool,
            tc.tile_pool(name="ring", bufs=1) as ring,
            tc.tile_pool(name="vp", bufs=4) as vp,
        ):
            pt = pool.tile([P, L, T], f32)      # exp'd emission probs per row
            x = pool.tile([P, L, T], f32)       # alpha trajectories
            ms = pool.tile([P, L], f32)         # DVE-owned mask copy
            zeros = pool.tile([P, T], f32)

            # col 0 of `zeros` is written by the pt-dependent TT below instead,
            # so that TT carries only the ACT wait (DVE ops have ONE wait slot)
            nc.vector.memset(zeros[:, 1:T], 0.0)
            nc.vector.memset(x[:, L - 1 : L, 0:1], 0.0)  # row 64 alpha0

            # stream log-probs in row chunks, exp per row on ACT. pt is only
            # ever ACT-written, so downstream DVE ops never need a DMA wait
            # (DVE instructions have a single sync-wait slot). The mask column
            # is copied to `ms` by one DVE copy per chunk, which absorbs the
            # chunk's DMA wait for the skip ops.
            for c0, c1 in IN_CHUNKS:
                lp_chunk = ring.tile([P, c1 - c0, T + 1], f32, tag=f"lp{c0}")
                nc.sync.dma_start(out=lp_chunk[:], in_=lp_in[:, c0:c1, :])
                nc.vector.tensor_copy(
                    ms[:, c0:c1], lp_chunk[:, :, T : T + 1]
                )
                for s in range(c0, c1):
                    nc.scalar.activation(
                        pt[:, s, :], lp_chunk[:, s - c0, 0:T], Act.Exp
                    )

            for c0 in range(0, L, CH):
                c1 = min(c0 + CH, L)  # chunk rows [c0, c1)
                # --- fence + alpha[t=0] init for this chunk ---
                if c0 == 0:
                    # zeros col 0 rewritten as a pt-dependent zero. Reading
                    # pt row 12 makes the first DVE op of the scan chain wait
                    # for ALL of chunk 0's exps, so every scan in the chunk
                    # inherits the ACT clock transitively (scan rows 0-2
                    # don't read the chunk fence below).
                    nc.vector.tensor_tensor(
                        out=zeros[:, 0:1],
                        in0=pt[:, CH - 1, 0:1],
                        in1=pt[:, CH - 1, 0:1],
                        op=Alu.subtract,
                    )
                    # alpha0 of rows 0,1 is pt[t=0]; carries the ACT>=2 wait
                    # that scan row 1 needs
                    nc.vector.tensor_copy(x[:, 0:2, 0:1], pt[:, 0:2, 0:1])
                    lo = 2
                else:
                    lo = c0 - 1
                # zero alpha[t=0] for rows [lo, c1-1): every scan row s in
                # this chunk reads row s-1 from exactly this range, so this TT
                # fences the chunk. Ranges of successive fences are disjoint
                # (no WAW) so each carries only its ACT wait. Reading this
                # chunk's last pt row makes it wait for all the chunk's exps.
                n = c1 - 1 - lo
                nc.vector.tensor_tensor(
                    out=x[:, lo : c1 - 1, 0:1],
                    in0=pt[:, c1 - n : c1, 0:1],
                    in1=pt[:, c1 - n : c1, 0:1],
                    op=Alu.subtract,
                )
                # --- scans for this chunk ---
                for s in range(c0, c1):
                    if s == 0:
                        data0 = zeros[:, 0 : T - 1]
                        init = pt[:, 0, 0:1]
                    elif s == 1:
                        data0 = x[:, 0, 0 : T - 1]
                        init = pt[:, 1, 0:1]
                    else:
                        init = 0.0
                        if s % 2 == 0 or modes[s] == "skip":
                            # blank rows (and odd rows whose skip is masked
                            # for every sample) take no skip transition
                            data0 = x[:, s - 1, 0 : T - 1]
                        elif modes[s] == "add":
                            # every sample allows the skip: plain add
                            v = vp.tile([P, T - 1], f32, tag="v")
                            nc.vector.tensor_tensor(
                                out=v[:],
                                in0=x[:, s - 2, 0 : T - 1],
                                in1=x[:, s - 1, 0 : T - 1],
                                op=Alu.add,
                            )
                            data0 = v[:]
                        else:
                            v = vp.tile([P, T - 1], f32, tag="v")
                            nc.vector.scalar_tensor_tensor(
                                out=v[:],
                                in0=x[:, s - 2, 0 : T - 1],
                                scalar=ms[:, s - 2 : s - 1],
                                in1=x[:, s - 1, 0 : T - 1],
                                op0=Alu.min,
                                op1=Alu.add,
                            )
                            data0 = v[:]
                    nc.vector.tensor_tensor_scan(
                        out=x[:, s, 1:T],
                        data0=data0,
                        data1=pt[:, s, 1:T],
                        initial=init,
                        op0=Alu.add,
                        op1=Alu.mult,
                    )
            # stream trajectories out in 4 chunks. Col 0 is skipped: the host
            # only reads t = il-1 >= 127, and skipping it keeps the out-DMA
            # read ranges disjoint from the col-0 fence writes.
            for o0, o1 in ((0, 17), (17, 33), (33, 49), (49, 65)):
                nc.sync.dma_start(out=x_out[:, o0:o1, 1:T], in_=x[:, o0:o1, 1:T])

    return nc


def _host_prep(log_probs, targets, input_lengths, target_lengths):
    lp = np.asarray(log_probs, dtype=np.float32)
    tg = np.asarray(targets).astype(np.int64)
    tl = np.asarray(target_lengths).astype(np.int64)

    j = np.arange(L)
    idx = np.clip(j // 2, 0, S - 1)
    odd = (j % 2) == 1
    valid = odd[None, :] & ((j // 2)[None, :] < tl[:, None])
    tp = np.where(valid, tg[:, idx], BLANK)  # (B, L)
    mask3 = tp[:, :-2] != tp[:, 2:]  # (B, L-2)

    # gather lp_tp[t, b, s] = lp[t, b, tp[b, s]] then lay out (B, L, T+1)
    # with the skip mask ({0, +inf}) packed into the final column
    g = lp[:, np.arange(B)[:, None], tp]  # (T, B, L)
    lp_tp = np.empty((B, L, T + 1), np.float32)
    lp_tp[:, :, :T] = g.transpose(1, 2, 0)

    msel = np.zeros((B, L), np.float32)
    msel[:, : L - 2] = np.where(mask3, np.float32(np.inf), np.float32(0.0))
    lp_tp[:, :, T] = msel
    return lp_tp, msel, tp


def kernel(log_probs, targets, input_lengths, target_lengths):
    from concourse.bass_utils import run_bass_kernel_spmd

    il = np.asarray(input_lengths).astype(np.int64)
    tl = np.asarray(target_lengths).astype(np.int64)
    lp_tp, msel, _ = _host_prep(log_probs, targets, input_lengths, target_lengths)

    # specialize odd rows by the dataset's skip-mask pattern (across ALL
    # samples, since one SPMD program serves every core)
    modes = ["n"] * L
    for s in range(3, L, 2):
        col = msel[:, s - 2]
        if np.all(np.isposinf(col)):
            modes[s] = "add"
        elif np.all(col == 0.0):
            modes[s] = "skip"
        else:
            modes[s] = "stt"
    key = "".join(m[0] for m in modes)
    if key not in _CACHE:
        _CACHE[key] = _build_program(modes)
    nc = _CACHE[key]

    in_maps = [
        {"lp_tp": np.ascontiguousarray(lp_tp[c * P : (c + 1) * P])}
        for c in range(N_CORES)
    ]
    res = run_bass_kernel_spmd(nc, in_maps, list(range(N_CORES)), trace=TRACE)
    global LAST_RESULT
    LAST_RESULT = res
    x = np.concatenate(
        [res.results[c]["x_out"] for c in range(N_CORES)], axis=0
    )  # (B, L, T)

    # readout: alpha at t = il-1 equals the reference's frozen final alpha
    bi = np.arange(B)
    af = x[bi[:, None], np.arange(L)[None, :], (il - 1)[:, None]]  # (B, L)
    end = 2 * tl
    ae = af[bi, end].astype(np.float32)
    ae1 = af[bi, end - 1].astype(np.float32)
    with np.errstate(divide="ignore", over="ignore"):
        loss = (-np.log(ae + ae1)).astype(np.float32)
        out = np.float32(np.mean((loss / tl.astype(np.float32)).astype(np.float32)))
    return np.array(out, dtype=np.float32)


# revision 40
# speedup vs baseline: 1.1294x; 1.0255x over previous
"""CTC loss (nn_CTCLoss) on 8 Trainium2 NeuronCores.

Strategy (data parallel over batch B, as per the sharding hint):
  - Host: build the extended-label table tp (blanks interleaved), gather the
    per-extended-label log-probs lp_tp[b, s, t] = log_probs[t, b, tp[b, s]],
    and the skip-transition mask. Shard batch across 8 cores (8 samples each).
  - Device (per core): exp on the ACT engine, then the T-step CTC forward DP.
    The DP is reorganized row-by-row over the 65 extended-label positions:
    each row is one hardware `tensor_tensor_scan` along the time axis
        state[t] = (cross[t-1] + state[t-1]) * pt[s, t]
    where cross = alpha[s-1] + where(mask, alpha[s-2], 0) comes from the two
    previously computed rows (one fused scalar_tensor_tensor op; the where is
    an exact min(x, {0,+inf}) per-partition trick, safe for inf without
    creating 0*inf NaNs). Instead of freezing alpha at t >= input_length, the
    full (unfrozen) trajectory x[b, s, t] is written out; the value at
    t = input_length-1 equals the reference's frozen final alpha.
  - Host: read alpha at t=il-1, loss = -log(a[2*tl] + a[2*tl-1]), mean(loss/tl).
"""

import numpy as np

T, B, C, S = 256, 64, 6000, 32
L = 2 * S + 1  # 65 extended labels (blank interleaved)
BLANK = 0
N_CORES = 8
P = B // N_CORES  # samples per core

_CACHE = {}

# test-harness knobs (harness calls kernel() with defaults untouched)
TRACE = False
LAST_RESULT = None


def _build_program(modes=None, read_s=None, read_t=None):
    import bass_rust
    import concourse.bass as bass
    import concourse.mybir as mybir
    from concourse import tile

    # modes[s] for odd rows s>=3: 'stt' = per-sample masked skip (min trick),
    # 'add' = all samples allow the skip (plain add), 'skip' = no sample
    # allows it (copy previous row only). Derived from the actual msel input
    # so the compiled program matches the dataset; defaults to the fully
    # general 'stt' everywhere.
    if modes is None:
        modes = ["stt"] * L
    # The host readout only needs alpha rows >= min(2*tl)-1 at times
    # >= min(il)-1, so only that slab of the trajectories is written out.
    # read_s = slab row start, read_t = slab time start.
    if read_s is None:
        read_s = 31
    if read_t is None:
        read_t = 127

    class SingleWaitTileContext(tile.TileContext):
        """TileContext whose epilogue drain never carries more than one sync
        wait: this target's codegen rejects any instruction with 2+ waits, and
        the stock epilogue puts the whole global clock on one drain. Absorb
        each outstanding semaphore tick with its own SP nop first."""

        def _drain_and_barrier(self, tick_clock, wait_clock):
            g = tick_clock.global_clock
            for proc in range(len(g)):
                if g[proc] > 0:
                    nop = self.nc.sync.nop()
                    pv = bass_rust.VectorClock()
                    pv.require_at_least(proc, g[proc])
                    wait_clock.add_sem_waits(
                        nop.ins, bass_rust.ScopedClock({None: pv})
                    )
            # stock epilogue, but with a wait-free drain: the nops above
            # already hold SP until the full global clock is reached
            self.nc.sync.drain()
            self.nc.all_engine_barrier()
            assert self.sems is not None
            popped = self.nc._tile_sem_poison_stack.pop()
            assert popped is self._sem_poison
            self.nc.clear_and_free_semaphores(
                list(self.sems.allocated().values())
            )
            self.nc.all_engine_barrier()

    f32 = mybir.dt.float32
    Alu = mybir.AluOpType
    Act = mybir.ActivationFunctionType

    nc = bass.Bass()
    # input rows are [log-probs(T) | msel] packed: col T carries the skip
    # mask as {0, +inf} so no separate mask DMA is needed
    lp_in = nc.dram_tensor("lp_tp", [P, L, T + 1], f32, kind="ExternalInput")
    OS, OT = L - read_s, T - read_t  # slab dims
    x_out = nc.dram_tensor("x_out", [P, OS, OT], f32, kind="ExternalOutput")
    # split point: everything up to row L-8 goes while late scans still run
    OSPLIT = max(read_s + 1, L - 8)

    CH = 5  # rows per compute (fence) chunk (65 = 13 * 5)
    IN_CHUNKS = ((0, 5), (5, 17), (17, 33), (33, 49), (49, 65))
    # 5 input + 1 output DMAs = 6 total: every DMA gets a fresh DMAHW lane
    # semaphore, and each input chunk has its own tile (no slot reuse), so
    # no DMA ever needs a second sync wait (every instruction on this
    # target has a single sync-wait slot). The first input chunk is small
    # so the scan chain starts early.

    # The TensorScalarPtr (scalar_tensor_tensor / tensor_tensor_scan) HW
    # encoding has a single sync-wait slot, so every cross-engine dependency
    # of those ops must be absorbed by a preceding non-TSP DVE instruction.
    # The col-0 "zero init" TensorTensor ops double as those fences: each
    # chunk's TT reads the chunk's exp'd pt rows (so it carries the ACT wait)
    # and writes the alpha[t=0] zeros that every scan in the chunk reads.
    with SingleWaitTileContext(nc) as tc:
        with (
            tc.tile_pool(name="main", bufs=1) as pool,
            tc.tile_pool(name="ring", bufs=1) as ring,
            tc.tile_pool(name="vp", bufs=4) as vp,
        ):
            pt = pool.tile([P, L, T], f32)      # exp'd emission probs per row
            x = pool.tile([P, L, T], f32)       # alpha trajectories
            ms = pool.tile([P, L], f32)         # DVE-owned mask copy
            zeros = pool.tile([P, T], f32)

            # col 0 of `zeros` is written by the pt-dependent TT below instead,
            # so that TT carries only the ACT wait (DVE ops have ONE wait slot)
            nc.vector.memset(zeros[:, 1:T], 0.0)
            nc.vector.memset(x[:, L - 1 : L, 0:1], 0.0)  # row 64 alpha0

            # stream log-probs in row chunks, exp per row on ACT. pt is only
            # ever ACT-written, so downstream DVE ops never need a DMA wait
            # (DVE instructions have a single sync-wait slot). The mask column
            # is copied to `ms` by one DVE copy per chunk, which absorbs the
            # chunk's DMA wait for the skip ops.
            for c0, c1 in IN_CHUNKS:
                lp_chunk = ring.tile([P, c1 - c0, T + 1], f32, tag=f"lp{c0}")
                nc.sync.dma_start(out=lp_chunk[:], in_=lp_in[:, c0:c1, :])
                nc.vector.tensor_copy(
                    ms[:, c0:c1], lp_chunk[:, :, T : T + 1]
                )
                for s in range(c0, c1):
                    nc.scalar.activation(
                        pt[:, s, :], lp_chunk[:, s - c0, 0:T], Act.Exp
                    )

            for c0 in range(0, L, CH):
                c1 = min(c0 + CH, L)  # chunk rows [c0, c1)
                # --- fence + alpha[t=0] init for this chunk ---
                if c0 == 0:
                    # zeros col 0 rewritten as a pt-dependent zero. Reading
                    # pt row 12 makes the first DVE op of the scan chain wait
                    # for ALL of chunk 0's exps, so every scan in the chunk
                    # inherits the ACT clock transitively (scan rows 0-2
                    # don't read the chunk fence below).
                    nc.vector.tensor_tensor(
                        out=zeros[:, 0:1],
                        in0=pt[:, CH - 1, 0:1],
                        in1=pt[:, CH - 1, 0:1],
                        op=Alu.subtract,
                    )
                    # alpha0 of rows 0,1 is pt[t=0]; carries the ACT>=2 wait
                    # that scan row 1 needs
                    nc.vector.tensor_copy(x[:, 0:2, 0:1], pt[:, 0:2, 0:1])
                    lo = 2
                else:
                    lo = c0 - 1
                # zero alpha[t=0] for rows [lo, c1-1): every scan row s in
                # this chunk reads row s-1 from exactly this range, so this TT
                # fences the chunk. Ranges of successive fences are disjoint
                # (no WAW) so each carries only its ACT wait. Reading this
                # chunk's last pt row makes it wait for all the chunk's exps.
                n = c1 - 1 - lo
                nc.vector.tensor_tensor(
                    out=x[:, lo : c1 - 1, 0:1],
                    in0=pt[:, c1 - n : c1, 0:1],
                    in1=pt[:, c1 - n : c1, 0:1],
                    op=Alu.subtract,
                )
                # --- scans for this chunk ---
                for s in range(c0, c1):
                    if s == 0:
                        data0 = zeros[:, 0 : T - 1]
                        init = pt[:, 0, 0:1]
                    elif s == 1:
                        data0 = x[:, 0, 0 : T - 1]
                        init = pt[:, 1, 0:1]
                    else:
                        init = 0.0
                        if s % 2 == 0 or modes[s] == "skip":
                            # blank rows (and odd rows whose skip is masked
                            # for every sample) take no skip transition
                            data0 = x[:, s - 1, 0 : T - 1]
                        elif modes[s] == "add":
                            # every sample allows the skip: plain add
                            v = vp.tile([P, T - 1], f32, tag="v")
                            nc.vector.tensor_tensor(
                                out=v[:],
                                in0=x[:, s - 2, 0 : T - 1],
                                in1=x[:, s - 1, 0 : T - 1],
                                op=Alu.add,
                            )
                            data0 = v[:]
                        else:
                            v = vp.tile([P, T - 1], f32, tag="v")
                            nc.vector.scalar_tensor_tensor(
                                out=v[:],
                                in0=x[:, s - 2, 0 : T - 1],
                                scalar=ms[:, s - 2 : s - 1],
                                in1=x[:, s - 1, 0 : T - 1],
                                op0=Alu.min,
                                op1=Alu.add,
                            )
                            data0 = v[:]
                    nc.vector.tensor_tensor_scan(
                        out=x[:, s, 1:T],
                        data0=data0,
                        data1=pt[:, s, 1:T],
                        initial=init,
                        op0=Alu.add,
                        op1=Alu.mult,
                    )
            # stream the readout slab out: a big chunk that overlaps the tail
            # scans, then a small final chunk right after the last scan
            nc.sync.dma_start(
                out=x_out[:, 0 : OSPLIT - read_s, :],
                in_=x[:, read_s:OSPLIT, read_t:T],
            )
            nc.sync.dma_start(
                out=x_out[:, OSPLIT - read_s : OS, :],
                in_=x[:, OSPLIT:L, read_t:T],
            )

    return nc


def _host_prep(log_probs, targets, input_lengths, target_lengths):
    lp = np.asarray(log_probs, dtype=np.float32)
    tg = np.asarray(targets).astype(np.int64)
    tl = np.asarray(target_lengths).astype(np.int64)

    j = np.arange(L)
    idx = np.clip(j // 2, 0, S - 1)
    odd = (j % 2) == 1
    valid = odd[None, :] & ((j // 2)[None, :] < tl[:, None])
    tp = np.where(valid, tg[:, idx], BLANK)  # (B, L)
    mask3 = tp[:, :-2] != tp[:, 2:]  # (B, L-2)

    # gather lp_tp[t, b, s] = lp[t, b, tp[b, s]] then lay out (B, L, T+1)
    # with the skip mask ({0, +inf}) packed into the final column
    g = lp[:, np.arange(B)[:, None], tp]  # (T, B, L)
    lp_tp = np.empty((B, L, T + 1), np.float32)
    lp_tp[:, :, :T] = g.transpose(1, 2, 0)

    msel = np.zeros((B, L), np.float32)
    msel[:, : L - 2] = np.where(mask3, np.float32(np.inf), np.float32(0.0))
    lp_tp[:, :, T] = msel
    return lp_tp, msel, tp


def kernel(log_probs, targets, input_lengths, target_lengths):
    from concourse.bass_utils import run_bass_kernel_spmd

    il = np.asarray(input_lengths).astype(np.int64)
    tl = np.asarray(target_lengths).astype(np.int64)
    lp_tp, msel, _ = _host_prep(log_probs, targets, input_lengths, target_lengths)

    # specialize odd rows by the dataset's skip-mask pattern (across ALL
    # samples, since one SPMD program serves every core)
    modes = ["n"] * L
    for s in range(3, L, 2):
        col = msel[:, s - 2]
        if np.all(np.isposinf(col)):
            modes[s] = "add"
        elif np.all(col == 0.0):
            modes[s] = "skip"
        else:
            modes[s] = "stt"
    end = 2 * tl
    read_s = max(1, int(end.min()) - 1)
    read_t = max(0, int(il.min()) - 1)
    key = ("".join(m[0] for m in modes), read_s, read_t)
    if key not in _CACHE:
        _CACHE[key] = _build_program(modes, read_s, read_t)
    nc = _CACHE[key]

    in_maps = [
        {"lp_tp": np.ascontiguousarray(lp_tp[c * P : (c + 1) * P])}
        for c in range(N_CORES)
    ]
    res = run_bass_kernel_spmd(nc, in_maps, list(range(N_CORES)), trace=TRACE)
    global LAST_RESULT
    LAST_RESULT = res
    xs = np.concatenate(
        [res.results[c]["x_out"] for c in range(N_CORES)], axis=0
    )  # (B, L-read_s, T-read_t)

    # alpha at t = il-1 equals the reference's frozen final alpha
    bi = np.arange(B)
    ae = xs[bi, end - read_s, il - 1 - read_t].astype(np.float32)
    ae1 = xs[bi, end - 1 - read_s, il - 1 - read_t].astype(np.float32)
    with np.errstate(divide="ignore", over="ignore"):
        loss = (-np.log(ae + ae1)).astype(np.float32)
        out = np.float32(np.mean((loss / tl.astype(np.float32)).astype(np.float32)))
    return np.array(out, dtype=np.float32)
